# revision 22
# baseline (speedup 1.0000x reference)
"""Multi-head attention (B=4, S=2048, D=1024, H=16, dk=64) on 8 TRN2 NeuronCores.

Sharding: core c = (batch b = c//2, head-group g = c%2 of 8 heads).
Each core computes its head-group's attention output and the partial output
projection (Wo rows for its heads); the host sums the two partials per batch
and adds the (folded) output bias.

Design (418us vs 474us baseline):
  - Everything bf16 on the PE (stationary + moving) -> FWL hides LDWEIGHTS,
    matmuls run at ~N*0.414ns at full p-state.
  - Scores psum is produced in the u' = 128*log2(e)*s domain (scale folded
    into Wq/bq on the host) so BOTH exp engines can consume it:
      * ScalarE activation Exp with scale=ln2/128 (free input affine)
      * a custom 8-stage DVE op EXP2_BITS_ANT writing uint16 bf16-BITS of
        2^(u'/128) directly: u2=u'-64; n1=(u2+M)-M (M=1.5*2^30: fp32 magic
        rounds to multiples of 128 -> floor); z=u2-n1 (=f1-64);
        out = u2 + (phi/128)*z^2 + (16320-32phi)
        (Schraudolph exp2 + parabolic mantissa correction; ~0.25% rms,
        ~0.1% after softmax normalization).
    Splitting exp 10/6 Scalar/DVE per head-pair removes the 273.7us
    single-engine exp wall that paced the baseline's attention phase; the
    attention loop then runs at the PE sequencer rate (~1.01us per skv
    tile = 4 matmuls + 4 ldweights).
  - AV weights padded to M=128 (vaug [skv,128]: V cols 0-63, ones col 64
    for denominators, junk cols 65-127 never read) keeping FWL on; the
    [128,512] f32 acc is still 1 PSUM bank.
  - y staged through ScalarE copies (Scalar has slack after the split).
  - Next-block Q-projection and prev-block out-projection are interleaved
    into the attention loop as fill chunks to keep the PE queue dense.
"""

import math

import numpy as np

B, S, D = 4, 2048, 1024
H, DK = 16, 64
LH = 8                 # heads per core
HK = LH * DK           # 512 (local concat dim)
BLK = 512              # Sq block size
NB = S // BLK          # 4
ST = S // 128          # 16 Skv tiles
KT = D // 128          # 8 contraction tiles over D
MT = HK // 128         # 4 m-tiles over local heads

# exp domain: psum holds u' = 128*log2(e) * scores
EXP_PHI = 0.34
EXP_C2 = 16320.0 - 32.0 * EXP_PHI
EXP_M = 1.5 * 2 ** 30
SCALAR_EXP_SCALE = math.log(2.0) / 128.0
U_SCALE = 128.0 / math.log(2.0)   # folded into Wq/bq (on top of 1/8)

# which i-tiles (per head-pair) run exp on the DVE instead of ScalarE.
# Spread so Scalar never runs more than 2 in a row: the two engines' exps
# must overlap or the exp chain paces the whole attention loop. Head-pairs
# 0-1 of each block also carry the out-projection psy->ysb copies on
# Scalar, so they shift more exp tiles to the DVE.
DVE_I_HEAVY = frozenset((1, 3, 5, 7, 9, 11, 13, 15))
DVE_I_LIGHT = frozenset((1, 3, 5, 7, 9, 11, 13))

_CACHE = {}


def _register_dve_exp():
    """Register the EXP2_BITS_ANT custom DVE op (documented extension point:
    dve_ops.OPS + sub-opcode row; rows 17..31 are free in the 5-bit field)."""
    import concourse.dve_ops as DOPS
    from concourse.dve_spec import C0, C1, C2, C3, Spec, Src0, lower, sq
    from concourse.dve_spec import _spill_c3_to_src1
    from concourse.dve_uop import DveOpSpec

    name = "EXP2_BITS_ANT"
    for op in DOPS.OPS:
        if op.name == name:
            return op

    u2 = Src0 - C0            # u' - 64
    h = u2 + C1               # + M  (fp32 rounds to a multiple of 128)
    n1 = h - C1               # 128*floor(u'/128)
    z = u2 - n1               # f1 - 64
    t = sq(z) * C3            # (phi/128) * z^2      (C3 via in1 spill)
    body = (u2 + t) + C2      # u' - 64 + t + (16320-32phi)

    def _ref(in0, in1, c0, c1, c2):
        f = np.float32
        u = in0.astype(f)
        u2 = (u - f(c0)).astype(f)
        h = (u2 + f(c1)).astype(f)
        n1 = (h - f(c1)).astype(f)
        z = (u2 - n1).astype(f)
        t = (z * z * np.asarray(in1, f).reshape(-1, 1)).astype(f)
        return ((u2 + t) + f(c2)).astype(f)

    spec = Spec(body=_spill_c3_to_src1(body), reference=_ref)
    row = max(DOPS._SUB_OPCODE_FOR_NAME.values()) + 1
    assert row < 0x20
    DOPS._SUB_OPCODE_FOR_NAME[name] = row
    uops = lower(spec, ver="v3")
    sha = DveOpSpec(name=name, opcode=row, uops=uops, rd1_en=True).sha("v3")
    op = DOPS.DveOp(name, spec, subdim=False, uops_sha={"v3": sha})
    DOPS.OPS.append(op)
    DOPS.CUSTOM_DVE_SPECS[name] = spec
    return op


def _build_program():
    from contextlib import ExitStack
    import concourse.tile as tile
    from concourse import bacc, mybir

    exp_op = _register_dve_exp()

    f32 = mybir.dt.float32
    bf16 = mybir.dt.bfloat16
    u16 = mybir.dt.uint16
    Exp = mybir.ActivationFunctionType.Exp

    nc = bacc.Bacc("TRN2", target_bir_lowering=False, debug=False, num_devices=8)

    xq_d = nc.dram_tensor("xq_t", [D, S], bf16, kind="ExternalInput")
    xk_d = nc.dram_tensor("xk_t", [D, S], bf16, kind="ExternalInput")
    xv_d = nc.dram_tensor("xv_t", [D, S], bf16, kind="ExternalInput")
    wq_d = nc.dram_tensor("wq", [D, HK], bf16, kind="ExternalInput")
    wk_d = nc.dram_tensor("wk", [D, HK], bf16, kind="ExternalInput")
    wv_d = nc.dram_tensor("wv", [D, HK], bf16, kind="ExternalInput")
    wo_d = nc.dram_tensor("wo", [HK, D], bf16, kind="ExternalInput")
    bq_d = nc.dram_tensor("bq2", [128, MT], f32, kind="ExternalInput")
    bk_d = nc.dram_tensor("bk2", [128, MT], f32, kind="ExternalInput")
    y_d = nc.dram_tensor("y_t", [D, S], f32, kind="ExternalOutput")

    with tile.TileContext(nc) as tc, ExitStack() as ctx:
        big = ctx.enter_context(tc.tile_pool(name="big", bufs=1))
        xs = ctx.enter_context(tc.tile_pool(name="xs", bufs=3))
        es_pool = ctx.enter_context(tc.tile_pool(name="es", bufs=12))
        ot_pool = ctx.enter_context(tc.tile_pool(name="ot", bufs=2))
        rpool = ctx.enter_context(tc.tile_pool(name="r", bufs=3))
        upool = ctx.enter_context(tc.tile_pool(name="u", bufs=3))
        ypool = ctx.enter_context(tc.tile_pool(name="y", bufs=3))
        # PSUM: psS 2x[128,1024] (4 banks) + psW 4x[128,512] (4 banks) = 8
        psS = ctx.enter_context(tc.tile_pool(name="psS", bufs=2, space="PSUM"))
        psW = ctx.enter_context(tc.tile_pool(name="psW", bufs=4, space="PSUM"))

        bq_sb = big.tile([128, MT], f32)
        bk_sb = big.tile([128, MT], f32)
        nc.sync.dma_start(bq_sb[:], bq_d[:])
        nc.sync.dma_start(bk_sb[:], bk_d[:])
        phi_sb = big.tile([128, 1], f32)
        nc.vector.memset(phi_sb[:], EXP_PHI / 128.0)

        qt = big.tile([128, MT, S], bf16)
        kt_ = big.tile([128, MT, S], bf16)
        # vaug[skv, st, h, 0:64] = V, col 64 = ones (denominator row), cols
        # 65:128 = junk-ones (never read; the pad keeps NumWeights==128 so
        # FWL stays enabled for the AV matmuls)
        vaug = big.tile([128, ST, LH, 128], bf16)
        nc.vector.memset(vaug[:, :, :, :].bitcast(u16), 0x3F80)

        # weight layouts keep each [128,128] matmul slice contiguous in the
        # innermost dim so Fast Weight Load stays enabled
        wq_sb = big.tile([128, MT, KT, 128], bf16, name="wq_sb")
        wk_sb = big.tile([128, MT, KT, 128], bf16, name="wk_sb")
        wv_sb = big.tile([128, KT, HK], bf16, name="wv_sb")
        wo_sb = big.tile([128, KT, MT, 128], bf16, name="wo_sb")

        W2 = 2 * BLK   # 1024-wide projection chains (2 Sq blocks per psum)

        def dma_x2(x_dram, half, tag, split=False):  # noqa: doc
            # one [128, KT, 1024] bf16 tile = half the sequence
            xt = xs.tile([128, KT, W2], bf16, tag="xs", name=f"xt_{tag}{half}")
            if split:  # stripe the cold-start tile across DMA queues
                for kt in range(KT):
                    nc.sync.dma_start(
                        xt[:, kt, :],
                        x_dram.ap()[kt * 128 : (kt + 1) * 128,
                                    half * W2 : (half + 1) * W2],
                    )
            else:
                nc.sync.dma_start(
                    xt[:],
                    x_dram.ap()[:, half * W2 : (half + 1) * W2]
                    .rearrange("(kt p) s -> p kt s", p=128),
                )
            return xt

        def proj_half(xt, w_sb, bias_sb, dst, half, mt, tag):
            # two N=512 chains sharing one [128,1024] psum tile (matmul
            # output must stay within one PSUM bank) + a single wide drain
            pp = psS.tile([128, W2], f32, tag="psS", name=f"pp_{tag}{half}_{mt}")
            for kt in range(KT):
                for h2 in range(2):
                    nc.tensor.matmul(
                        pp[:, h2 * BLK : (h2 + 1) * BLK],
                        w_sb[:, mt, kt, :],
                        xt[:, kt, h2 * BLK : (h2 + 1) * BLK],
                        start=(kt == 0),
                        stop=(kt == KT - 1),
                        skip_group_check=True,
                    )
            nc.vector.tensor_scalar_add(
                dst[:, mt, half * W2 : (half + 1) * W2], pp[:],
                bias_sb[:, mt : mt + 1],
            )

        # ---- K projection, full Q projection, V projection (N=1024) ----
        xk0 = dma_x2(xk_d, 0, "k", split=True)
        wk_re = wk_d.ap().rearrange("(kt p) (mt m) -> p mt kt m", p=128, mt=MT, m=128)
        for mt in range(MT):
            nc.sync.dma_start(wk_sb[:, mt], wk_re[:, mt])
        xk1 = dma_x2(xk_d, 1, "k")
        nc.sync.dma_start(
            wq_sb[:],
            wq_d.ap().rearrange("(kt p) (mt m) -> p mt kt m", p=128, mt=MT, m=128),
        )
        for mt in range(MT):
            proj_half(xk0, wk_sb, bk_sb, kt_, 0, mt, "k")
        xq0 = dma_x2(xq_d, 0, "q")
        for mt in range(MT):
            proj_half(xk1, wk_sb, bk_sb, kt_, 1, mt, "k")
        xq1 = dma_x2(xq_d, 1, "q")
        nc.sync.dma_start(wv_sb[:], wv_d.ap().rearrange("(kt p) m -> p kt m", p=128))
        for mt in range(MT):
            proj_half(xq0, wq_sb, bq_sb, qt, 0, mt, "q")
        xv0 = dma_x2(xv_d, 0, "v")
        for mt in range(MT):
            proj_half(xq1, wq_sb, bq_sb, qt, 1, mt, "q")
        nc.sync.dma_start(
            wo_sb[:],
            wo_d.ap().rearrange("(kt p) (mo m) -> p mo kt m", p=128, kt=MT, m=128),
        )
        xv1 = dma_x2(xv_d, 1, "v")
        for half, xtv in ((0, xv0), (1, xv1)):
            for qp in range(4):
                pp = [psW.tile([128, BLK], f32, tag="psW",
                               name=f"pp_v{half}_{qp}_{t}") for t in range(2)]
                for kt in range(KT):
                    for t in range(2):
                        q = 2 * qp + t
                        nc.tensor.matmul(
                            pp[t][:],
                            xtv[:, kt, q * 128 : (q + 1) * 128],
                            wv_sb[:, kt, :],
                            start=(kt == 0),
                            stop=(kt == KT - 1),
                            skip_group_check=True,
                        )
                for t in range(2):
                    st = half * 8 + 2 * qp + t
                    nc.vector.tensor_copy(
                        vaug[:, st, :, 0:DK],
                        pp[t][:].rearrange("p (h k) -> p h k", h=LH),
                    )

        # ---- fill queue: PE chunks interleaved into the attention loop ----
        fills = []

        def emit_outproj_chunks(jprev, ot_prev):
            for mo in range(KT):
                def op_(mo=mo, jprev=jprev, ot_prev=ot_prev):
                    psy = psW.tile([128, BLK], f32, tag="psW",
                                   name=f"psy{jprev}_{mo}")
                    for kt in range(MT):
                        nc.tensor.matmul(
                            psy[:],
                            wo_sb[:, mo, kt, :],
                            ot_prev[:, kt, :],
                            start=(kt == 0),
                            stop=(kt == MT - 1),
                            skip_group_check=True,
                        )
                    ysb = ypool.tile([128, BLK], f32, tag="y",
                                     name=f"ysb{jprev}_{mo}")
                    if mo % 2 == 0:
                        nc.scalar.copy(ysb[:], psy[:])
                    else:
                        nc.vector.tensor_copy(ysb[:], psy[:])
                    if jprev == NB - 1:
                        for hy in range(2):
                            nc.sync.dma_start(
                                y_d[mo * 128 : (mo + 1) * 128,
                                    jprev * BLK + hy * 256
                                    : jprev * BLK + (hy + 1) * 256],
                                ysb[:, hy * 256 : (hy + 1) * 256],
                            )
                    else:
                        nc.sync.dma_start(
                            y_d[mo * 128 : (mo + 1) * 128,
                                jprev * BLK : (jprev + 1) * BLK], ysb[:]
                        )
                fills.append(op_)

        def fill(n):
            for _ in range(n):
                if fills:
                    fills.pop(0)()

        # ---- attention: per block, per head-pair, i-major; AV one-late ----
        ot_prev = None
        for j in range(NB):
            otj = ot_pool.tile([128, MT, BLK], bf16)
            if j >= 1:
                emit_outproj_chunks(j - 1, ot_prev)
            for hp in range(LH // 2):
                mt = hp
                acc = []

                def emit_av(iu, es_t, acc=acc, hp=hp, j=j):
                    if not acc:
                        acc.extend(psW.tile([128, BLK], f32, tag="psW",
                                            name=f"acc{j}_{hp}_{p2}")
                                   for p2 in range(2))
                    for pi in range(2):
                        h = 2 * hp + pi
                        nc.tensor.matmul(
                            acc[pi][:],
                            vaug[:, iu, h, :],
                            es_t[:, pi * BLK : (pi + 1) * BLK],
                            start=(iu == 0),
                            stop=(iu == ST - 1),
                            skip_group_check=True,
                        )

                # i-tiles processed in PAIRS: both scores pairs (64x128
                # row-tiled mode) back-to-back, then a batch of AV matmuls
                # (128x128 mode) — halves the PE tiling-mode switch drains
                es_q = []
                for i in range(ST):
                    ps2 = psS.tile([128, 2 * BLK], f32, tag="psS",
                                   name=f"ps_s{j}_{hp}_{i}")
                    for pi in range(2):
                        bp = pi * 64
                        nc.tensor.matmul(
                            ps2[:, pi * BLK : (pi + 1) * BLK],
                            kt_[bp : bp + 64, mt, i * 128 : (i + 1) * 128],
                            qt[bp : bp + 64, mt, j * BLK : (j + 1) * BLK],
                            start=True,
                            stop=True,
                            skip_group_check=True,
                        )
                    es = es_pool.tile([128, 2 * BLK], bf16, tag="es")
                    if i in DVE_I_LIGHT:
                        nc.vector._custom_dve(
                            exp_op, out=es[:].bitcast(u16), in0=ps2[:],
                            in1=phi_sb[:, 0:1], s0=64.0, s1=EXP_M, imm2=EXP_C2,
                        )
                    else:
                        nc.scalar.activation(es[:], ps2[:], Exp,
                                             scale=SCALAR_EXP_SCALE)
                    es_q.append((i, es))
                    if i % 2 == 1:
                        while len(es_q) > 2:
                            iu, es_t = es_q.pop(0)
                            emit_av(iu, es_t)
                        if i in (5, 9):
                            fill(1)
                for iu, es_t in es_q:
                    emit_av(iu, es_t)
                def emit_norm(acc=acc, mt=mt, hp=hp, j=j, otj=otj):
                    rrow, rf, rbc = [], [], []
                    for pi in range(2):
                        rrow.append(rpool.tile([1, BLK], f32, tag="r",
                                               name=f"rr{j}_{hp}_{pi}"))
                        nc.vector.tensor_copy(rrow[pi][:],
                                              acc[pi][DK : DK + 1, :])
                    for pi in range(2):
                        rf.append(rpool.tile([1, BLK], f32, tag="rf",
                                             name=f"rf{j}_{hp}_{pi}"))
                        nc.vector.reciprocal_approx_fast(rf[pi][:], rrow[pi][:])
                    for pi in range(2):
                        rbc.append(upool.tile([DK, BLK], f32, tag="rb",
                                              name=f"rb{j}_{hp}_{pi}"))
                        nc.gpsimd.partition_broadcast(rbc[pi][:], rf[pi][:])
                    for pi in range(2):
                        nc.vector.tensor_mul(otj[pi * 64 : pi * 64 + 64, mt, :],
                                             acc[pi][0:DK, :], rbc[pi][:])
                emit_norm()
            fill(len(fills))
            ot_prev = otj
        emit_outproj_chunks(NB - 1, ot_prev)
        fill(len(fills))

    nc.compile()
    return nc


def get_program():
    if "nc" not in _CACHE:
        _CACHE["nc"] = _build_program()
    return _CACHE["nc"]


def make_core_inputs(query, key, value, Wq, bq, Wk, bk, Wv, bv, Wo, bo):
    """Build the 8 per-core input dicts (and the folded output bias)."""
    import ml_dtypes
    f = np.float32
    bf = ml_dtypes.bfloat16
    # scores scale 1/8 and the exp2 domain 128*log2(e) folded into Wq/bq
    qs = U_SCALE / 8.0
    in_maps = []
    for c in range(8):
        b, g = c // 2, c % 2
        hs = slice(g * LH, (g + 1) * LH)
        m = {
            "xq_t": np.ascontiguousarray(query[b].T).astype(bf),
            "xk_t": np.ascontiguousarray(key[b].T).astype(bf),
            "xv_t": np.ascontiguousarray(value[b].T).astype(bf),
            "wq": np.ascontiguousarray(
                Wq[hs].transpose(1, 0, 2).reshape(D, HK) * qs
            ).astype(bf),
            "wk": np.ascontiguousarray(
                Wk[hs].transpose(1, 0, 2).reshape(D, HK)
            ).astype(bf),
            "wv": np.ascontiguousarray(
                Wv[hs].transpose(1, 0, 2).reshape(D, HK)
            ).astype(bf),
            "wo": np.ascontiguousarray(Wo[g * HK : (g + 1) * HK, :]).astype(bf),
            "bq2": np.ascontiguousarray(
                (bq[hs].reshape(HK) * qs).reshape(MT, 128).T, dtype=f
            ),
            "bk2": np.ascontiguousarray(
                bk[hs].reshape(HK).reshape(MT, 128).T, dtype=f
            ),
        }
        in_maps.append(m)
    bo_eff = (bv.reshape(H * DK).astype(np.float64) @ Wo.astype(np.float64)
              + bo.astype(np.float64)).astype(f)
    return in_maps, bo_eff


def combine_outputs(results, bo_eff):
    """results: list of 8 dicts with 'y_t' [D, S]. Returns [B, S, D] f32."""
    out = np.empty((B, S, D), dtype=np.float32)
    for b in range(B):
        acc = results[2 * b]["y_t"] + results[2 * b + 1]["y_t"]
        out[b] = acc.T + bo_eff[None, :]
    return out


def kernel(**inputs):
    from concourse.bass_utils import run_bass_kernel_spmd

    inputs = {k: np.asarray(v) for k, v in inputs.items()}
    nc = get_program()
    in_maps, bo_eff = make_core_inputs(
        inputs["query"], inputs["key"], inputs["value"],
        inputs["Wq"], inputs["bq"], inputs["Wk"], inputs["bk"],
        inputs["Wv"], inputs["bv"], inputs["Wo"], inputs["bo"],
    )
    res = run_bass_kernel_spmd(nc, in_maps, list(range(8)))
    return combine_outputs(res.results, bo_eff)


# revision 23
# speedup vs baseline: 1.0156x; 1.0156x over previous
"""Multi-head attention (B=4, S=2048, D=1024, H=16, dk=64) on 8 TRN2 NeuronCores.

Sharding: core c = (batch b = c//2, head-group g = c%2 of 8 heads).
Each core computes its head-group's attention output and the partial output
projection (Wo rows for its heads); the host sums the two partials per batch
and adds the (folded) output bias.

Design (418us vs 474us baseline):
  - Everything bf16 on the PE (stationary + moving) -> FWL hides LDWEIGHTS,
    matmuls run at ~N*0.414ns at full p-state.
  - Scores psum is produced in the u' = 128*log2(e)*s domain (scale folded
    into Wq/bq on the host) so BOTH exp engines can consume it:
      * ScalarE activation Exp with scale=ln2/128 (free input affine)
      * a custom 8-stage DVE op EXP2_BITS_ANT writing uint16 bf16-BITS of
        2^(u'/128) directly: u2=u'-64; n1=(u2+M)-M (M=1.5*2^30: fp32 magic
        rounds to multiples of 128 -> floor); z=u2-n1 (=f1-64);
        out = u2 + (phi/128)*z^2 + (16320-32phi)
        (Schraudolph exp2 + parabolic mantissa correction; ~0.25% rms,
        ~0.1% after softmax normalization).
    Splitting exp 10/6 Scalar/DVE per head-pair removes the 273.7us
    single-engine exp wall that paced the baseline's attention phase; the
    attention loop then runs at the PE sequencer rate (~1.01us per skv
    tile = 4 matmuls + 4 ldweights).
  - AV weights padded to M=128 (vaug [skv,128]: V cols 0-63, ones col 64
    for denominators, junk cols 65-127 never read) keeping FWL on; the
    [128,512] f32 acc is still 1 PSUM bank.
  - y staged through ScalarE copies (Scalar has slack after the split).
  - Next-block Q-projection and prev-block out-projection are interleaved
    into the attention loop as fill chunks to keep the PE queue dense.
"""

import math

import numpy as np

B, S, D = 4, 2048, 1024
H, DK = 16, 64
LH = 8                 # heads per core
HK = LH * DK           # 512 (local concat dim)
BLK = 512              # Sq block size
NB = S // BLK          # 4
ST = S // 128          # 16 Skv tiles
KT = D // 128          # 8 contraction tiles over D
MT = HK // 128         # 4 m-tiles over local heads

# exp domain: psum holds u' = 128*log2(e) * scores
EXP_PHI = 0.34
EXP_C2 = 16320.0 - 32.0 * EXP_PHI
EXP_M = 1.5 * 2 ** 30
SCALAR_EXP_SCALE = math.log(2.0) / 128.0
U_SCALE = 128.0 / math.log(2.0)   # folded into Wq/bq (on top of 1/8)

# which i-tiles (per head-pair) run exp on the DVE instead of ScalarE.
# Spread so Scalar never runs more than 2 in a row: the two engines' exps
# must overlap or the exp chain paces the whole attention loop. Head-pairs
# 0-1 of each block also carry the out-projection psy->ysb copies on
# Scalar, so they shift more exp tiles to the DVE.
DVE_I_HEAVY = frozenset((1, 3, 5, 7, 9, 11, 13, 15))
DVE_I_LIGHT = frozenset((1, 3, 6, 9, 11, 13))

_CACHE = {}


def _register_dve_exp():
    """Register the EXP2_BITS_ANT custom DVE op (documented extension point:
    dve_ops.OPS + sub-opcode row; rows 17..31 are free in the 5-bit field)."""
    import concourse.dve_ops as DOPS
    from concourse.dve_spec import C0, C1, C2, C3, Spec, Src0, lower, sq
    from concourse.dve_spec import _spill_c3_to_src1
    from concourse.dve_uop import DveOpSpec

    name = "EXP2_BITS_ANT"
    for op in DOPS.OPS:
        if op.name == name:
            return op

    u2 = Src0 - C0            # u' - 64
    h = u2 + C1               # + M  (fp32 rounds to a multiple of 128)
    n1 = h - C1               # 128*floor(u'/128)
    z = u2 - n1               # f1 - 64
    t = sq(z) * C3            # (phi/128) * z^2      (C3 via in1 spill)
    body = (u2 + t) + C2      # u' - 64 + t + (16320-32phi)

    def _ref(in0, in1, c0, c1, c2):
        f = np.float32
        u = in0.astype(f)
        u2 = (u - f(c0)).astype(f)
        h = (u2 + f(c1)).astype(f)
        n1 = (h - f(c1)).astype(f)
        z = (u2 - n1).astype(f)
        t = (z * z * np.asarray(in1, f).reshape(-1, 1)).astype(f)
        return ((u2 + t) + f(c2)).astype(f)

    spec = Spec(body=_spill_c3_to_src1(body), reference=_ref)
    row = max(DOPS._SUB_OPCODE_FOR_NAME.values()) + 1
    assert row < 0x20
    DOPS._SUB_OPCODE_FOR_NAME[name] = row
    uops = lower(spec, ver="v3")
    sha = DveOpSpec(name=name, opcode=row, uops=uops, rd1_en=True).sha("v3")
    op = DOPS.DveOp(name, spec, subdim=False, uops_sha={"v3": sha})
    DOPS.OPS.append(op)
    DOPS.CUSTOM_DVE_SPECS[name] = spec
    return op


def _build_program():
    from contextlib import ExitStack
    import concourse.tile as tile
    from concourse import bacc, mybir

    exp_op = _register_dve_exp()

    f32 = mybir.dt.float32
    bf16 = mybir.dt.bfloat16
    u16 = mybir.dt.uint16
    Exp = mybir.ActivationFunctionType.Exp

    nc = bacc.Bacc("TRN2", target_bir_lowering=False, debug=False, num_devices=8)

    xq_d = nc.dram_tensor("xq_t", [D, S], bf16, kind="ExternalInput")
    xk_d = nc.dram_tensor("xk_t", [D, S], bf16, kind="ExternalInput")
    xv_d = nc.dram_tensor("xv_t", [D, S], bf16, kind="ExternalInput")
    wq_d = nc.dram_tensor("wq", [D, HK], bf16, kind="ExternalInput")
    wk_d = nc.dram_tensor("wk", [D, HK], bf16, kind="ExternalInput")
    wv_d = nc.dram_tensor("wv", [D, HK], bf16, kind="ExternalInput")
    wo_d = nc.dram_tensor("wo", [HK, D], bf16, kind="ExternalInput")
    bq_d = nc.dram_tensor("bq2", [128, MT], f32, kind="ExternalInput")
    bk_d = nc.dram_tensor("bk2", [128, MT], f32, kind="ExternalInput")
    y_d = nc.dram_tensor("y_t", [D, S], f32, kind="ExternalOutput")

    with tile.TileContext(nc) as tc, ExitStack() as ctx:
        big = ctx.enter_context(tc.tile_pool(name="big", bufs=1))
        xs = ctx.enter_context(tc.tile_pool(name="xs", bufs=3))
        es_pool = ctx.enter_context(tc.tile_pool(name="es", bufs=12))
        ot_pool = ctx.enter_context(tc.tile_pool(name="ot", bufs=2))
        rpool = ctx.enter_context(tc.tile_pool(name="r", bufs=3))
        upool = ctx.enter_context(tc.tile_pool(name="u", bufs=3))
        ypool = ctx.enter_context(tc.tile_pool(name="y", bufs=3))
        # PSUM: psS 2x[128,1024] (4 banks) + psW 4x[128,512] (4 banks) = 8
        psS = ctx.enter_context(tc.tile_pool(name="psS", bufs=2, space="PSUM"))
        psW = ctx.enter_context(tc.tile_pool(name="psW", bufs=4, space="PSUM"))

        bq_sb = big.tile([128, MT], f32)
        bk_sb = big.tile([128, MT], f32)
        nc.sync.dma_start(bq_sb[:], bq_d[:])
        nc.sync.dma_start(bk_sb[:], bk_d[:])
        phi_sb = big.tile([128, 1], f32)
        nc.vector.memset(phi_sb[:], EXP_PHI / 128.0)

        qt = big.tile([128, MT, S], bf16)
        kt_ = big.tile([128, MT, S], bf16)
        # vaug[skv, st, h, 0:64] = V, col 64 = ones (denominator row), cols
        # 65:128 = junk-ones (never read; the pad keeps NumWeights==128 so
        # FWL stays enabled for the AV matmuls)
        vaug = big.tile([128, ST, LH, 128], bf16)
        nc.vector.memset(vaug[:, :, :, :].bitcast(u16), 0x3F80)

        # weight layouts keep each [128,128] matmul slice contiguous in the
        # innermost dim so Fast Weight Load stays enabled
        wq_sb = big.tile([128, MT, KT, 128], bf16, name="wq_sb")
        wk_sb = big.tile([128, MT, KT, 128], bf16, name="wk_sb")
        wv_sb = big.tile([128, KT, HK], bf16, name="wv_sb")
        wo_sb = big.tile([128, KT, MT, 128], bf16, name="wo_sb")

        W2 = 2 * BLK   # 1024-wide projection chains (2 Sq blocks per psum)

        def dma_x2(x_dram, half, tag, split=False):  # noqa: doc
            # one [128, KT, 1024] bf16 tile = half the sequence
            xt = xs.tile([128, KT, W2], bf16, tag="xs", name=f"xt_{tag}{half}")
            if split:  # stripe the cold-start tile across DMA queues
                for kt in range(KT):
                    nc.sync.dma_start(
                        xt[:, kt, :],
                        x_dram.ap()[kt * 128 : (kt + 1) * 128,
                                    half * W2 : (half + 1) * W2],
                    )
            else:
                nc.sync.dma_start(
                    xt[:],
                    x_dram.ap()[:, half * W2 : (half + 1) * W2]
                    .rearrange("(kt p) s -> p kt s", p=128),
                )
            return xt

        def proj_half(xt, w_sb, bias_sb, dst, half, mt, tag):
            # two N=512 chains sharing one [128,1024] psum tile (matmul
            # output must stay within one PSUM bank) + a single wide drain
            pp = psS.tile([128, W2], f32, tag="psS", name=f"pp_{tag}{half}_{mt}")
            for kt in range(KT):
                for h2 in range(2):
                    nc.tensor.matmul(
                        pp[:, h2 * BLK : (h2 + 1) * BLK],
                        w_sb[:, mt, kt, :],
                        xt[:, kt, h2 * BLK : (h2 + 1) * BLK],
                        start=(kt == 0),
                        stop=(kt == KT - 1),
                        skip_group_check=True,
                    )
            nc.vector.tensor_scalar_add(
                dst[:, mt, half * W2 : (half + 1) * W2], pp[:],
                bias_sb[:, mt : mt + 1],
            )

        # ---- K projection, full Q projection, V projection (N=1024) ----
        xk0 = dma_x2(xk_d, 0, "k", split=True)
        wk_re = wk_d.ap().rearrange("(kt p) (mt m) -> p mt kt m", p=128, mt=MT, m=128)
        for mt in range(MT):
            nc.sync.dma_start(wk_sb[:, mt], wk_re[:, mt])
        xk1 = dma_x2(xk_d, 1, "k")
        nc.sync.dma_start(
            wq_sb[:],
            wq_d.ap().rearrange("(kt p) (mt m) -> p mt kt m", p=128, mt=MT, m=128),
        )
        for mt in range(MT):
            proj_half(xk0, wk_sb, bk_sb, kt_, 0, mt, "k")
        xq0 = dma_x2(xq_d, 0, "q")
        for mt in range(MT):
            proj_half(xk1, wk_sb, bk_sb, kt_, 1, mt, "k")
        xq1 = dma_x2(xq_d, 1, "q")
        nc.sync.dma_start(wv_sb[:], wv_d.ap().rearrange("(kt p) m -> p kt m", p=128))
        for mt in range(MT):
            proj_half(xq0, wq_sb, bq_sb, qt, 0, mt, "q")
        xv0 = dma_x2(xv_d, 0, "v")
        for mt in range(MT):
            proj_half(xq1, wq_sb, bq_sb, qt, 1, mt, "q")
        nc.sync.dma_start(
            wo_sb[:],
            wo_d.ap().rearrange("(kt p) (mo m) -> p mo kt m", p=128, kt=MT, m=128),
        )
        xv1 = dma_x2(xv_d, 1, "v")
        for half, xtv in ((0, xv0), (1, xv1)):
            for qp in range(4):
                pp = [psW.tile([128, BLK], f32, tag="psW",
                               name=f"pp_v{half}_{qp}_{t}") for t in range(2)]
                for kt in range(KT):
                    for t in range(2):
                        q = 2 * qp + t
                        nc.tensor.matmul(
                            pp[t][:],
                            xtv[:, kt, q * 128 : (q + 1) * 128],
                            wv_sb[:, kt, :],
                            start=(kt == 0),
                            stop=(kt == KT - 1),
                            skip_group_check=True,
                        )
                for t in range(2):
                    st = half * 8 + 2 * qp + t
                    nc.vector.tensor_copy(
                        vaug[:, st, :, 0:DK],
                        pp[t][:].rearrange("p (h k) -> p h k", h=LH),
                    )

        # ---- fill queue: PE chunks interleaved into the attention loop ----
        fills = []

        def emit_outproj_chunks(jprev, ot_prev):
            for mo in range(KT):
                def op_(mo=mo, jprev=jprev, ot_prev=ot_prev):
                    psy = psW.tile([128, BLK], f32, tag="psW",
                                   name=f"psy{jprev}_{mo}")
                    for kt in range(MT):
                        nc.tensor.matmul(
                            psy[:],
                            wo_sb[:, mo, kt, :],
                            ot_prev[:, kt, :],
                            start=(kt == 0),
                            stop=(kt == MT - 1),
                            skip_group_check=True,
                        )
                    ysb = ypool.tile([128, BLK], f32, tag="y",
                                     name=f"ysb{jprev}_{mo}")
                    if mo % 2 == 0:
                        nc.scalar.copy(ysb[:], psy[:])
                    else:
                        nc.vector.tensor_copy(ysb[:], psy[:])
                    if jprev == NB - 1:
                        for hy in range(2):
                            nc.sync.dma_start(
                                y_d[mo * 128 : (mo + 1) * 128,
                                    jprev * BLK + hy * 256
                                    : jprev * BLK + (hy + 1) * 256],
                                ysb[:, hy * 256 : (hy + 1) * 256],
                            )
                    else:
                        nc.sync.dma_start(
                            y_d[mo * 128 : (mo + 1) * 128,
                                jprev * BLK : (jprev + 1) * BLK], ysb[:]
                        )
                fills.append(op_)

        def fill(n):
            for _ in range(n):
                if fills:
                    fills.pop(0)()

        # ---- attention: per block, per head-pair, i-major; AV one-late ----
        ot_prev = None
        for j in range(NB):
            otj = ot_pool.tile([128, MT, BLK], bf16)
            if j >= 1:
                emit_outproj_chunks(j - 1, ot_prev)
            for hp in range(LH // 2):
                mt = hp
                acc = []

                def emit_av(iu, es_t, acc=acc, hp=hp, j=j):
                    if not acc:
                        acc.extend(psW.tile([128, BLK], f32, tag="psW",
                                            name=f"acc{j}_{hp}_{p2}")
                                   for p2 in range(2))
                    for pi in range(2):
                        h = 2 * hp + pi
                        nc.tensor.matmul(
                            acc[pi][:],
                            vaug[:, iu, h, :],
                            es_t[:, pi * BLK : (pi + 1) * BLK],
                            start=(iu == 0),
                            stop=(iu == ST - 1),
                            skip_group_check=True,
                        )

                # i-tiles processed in PAIRS: both scores pairs (64x128
                # row-tiled mode) back-to-back, then a batch of AV matmuls
                # (128x128 mode) — halves the PE tiling-mode switch drains
                es_q = []
                for i in range(ST):
                    ps2 = psS.tile([128, 2 * BLK], f32, tag="psS",
                                   name=f"ps_s{j}_{hp}_{i}")
                    for pi in range(2):
                        bp = pi * 64
                        nc.tensor.matmul(
                            ps2[:, pi * BLK : (pi + 1) * BLK],
                            kt_[bp : bp + 64, mt, i * 128 : (i + 1) * 128],
                            qt[bp : bp + 64, mt, j * BLK : (j + 1) * BLK],
                            start=True,
                            stop=True,
                            skip_group_check=True,
                        )
                    es = es_pool.tile([128, 2 * BLK], bf16, tag="es")
                    if i in DVE_I_LIGHT:
                        nc.vector._custom_dve(
                            exp_op, out=es[:].bitcast(u16), in0=ps2[:],
                            in1=phi_sb[:, 0:1], s0=64.0, s1=EXP_M, imm2=EXP_C2,
                        )
                    else:
                        nc.scalar.activation(es[:], ps2[:], Exp,
                                             scale=SCALAR_EXP_SCALE)
                    es_q.append((i, es))
                    if i % 2 == 1:
                        while len(es_q) > 2:
                            iu, es_t = es_q.pop(0)
                            emit_av(iu, es_t)
                        if i in (5, 9):
                            fill(1)
                for iu, es_t in es_q:
                    emit_av(iu, es_t)
                def emit_norm(acc=acc, mt=mt, hp=hp, j=j, otj=otj):
                    rrow, rf, rbc = [], [], []
                    for pi in range(2):
                        rrow.append(rpool.tile([1, BLK], f32, tag="r",
                                               name=f"rr{j}_{hp}_{pi}"))
                        nc.vector.tensor_copy(rrow[pi][:],
                                              acc[pi][DK : DK + 1, :])
                    for pi in range(2):
                        rf.append(rpool.tile([1, BLK], f32, tag="rf",
                                             name=f"rf{j}_{hp}_{pi}"))
                        nc.vector.reciprocal_approx_fast(rf[pi][:], rrow[pi][:])
                    for pi in range(2):
                        rbc.append(upool.tile([DK, BLK], f32, tag="rb",
                                              name=f"rb{j}_{hp}_{pi}"))
                        nc.gpsimd.partition_broadcast(rbc[pi][:], rf[pi][:])
                    for pi in range(2):
                        nc.vector.tensor_mul(otj[pi * 64 : pi * 64 + 64, mt, :],
                                             acc[pi][0:DK, :], rbc[pi][:])
                emit_norm()
            fill(len(fills))
            ot_prev = otj
        emit_outproj_chunks(NB - 1, ot_prev)
        fill(len(fills))

    nc.compile()
    return nc


def get_program():
    if "nc" not in _CACHE:
        _CACHE["nc"] = _build_program()
    return _CACHE["nc"]


def make_core_inputs(query, key, value, Wq, bq, Wk, bk, Wv, bv, Wo, bo):
    """Build the 8 per-core input dicts (and the folded output bias)."""
    import ml_dtypes
    f = np.float32
    bf = ml_dtypes.bfloat16
    # scores scale 1/8 and the exp2 domain 128*log2(e) folded into Wq/bq
    qs = U_SCALE / 8.0
    in_maps = []
    for c in range(8):
        b, g = c // 2, c % 2
        hs = slice(g * LH, (g + 1) * LH)
        m = {
            "xq_t": np.ascontiguousarray(query[b].T).astype(bf),
            "xk_t": np.ascontiguousarray(key[b].T).astype(bf),
            "xv_t": np.ascontiguousarray(value[b].T).astype(bf),
            "wq": np.ascontiguousarray(
                Wq[hs].transpose(1, 0, 2).reshape(D, HK) * qs
            ).astype(bf),
            "wk": np.ascontiguousarray(
                Wk[hs].transpose(1, 0, 2).reshape(D, HK)
            ).astype(bf),
            "wv": np.ascontiguousarray(
                Wv[hs].transpose(1, 0, 2).reshape(D, HK)
            ).astype(bf),
            "wo": np.ascontiguousarray(Wo[g * HK : (g + 1) * HK, :]).astype(bf),
            "bq2": np.ascontiguousarray(
                (bq[hs].reshape(HK) * qs).reshape(MT, 128).T, dtype=f
            ),
            "bk2": np.ascontiguousarray(
                bk[hs].reshape(HK).reshape(MT, 128).T, dtype=f
            ),
        }
        in_maps.append(m)
    bo_eff = (bv.reshape(H * DK).astype(np.float64) @ Wo.astype(np.float64)
              + bo.astype(np.float64)).astype(f)
    return in_maps, bo_eff


def combine_outputs(results, bo_eff):
    """results: list of 8 dicts with 'y_t' [D, S]. Returns [B, S, D] f32."""
    out = np.empty((B, S, D), dtype=np.float32)
    for b in range(B):
        acc = results[2 * b]["y_t"] + results[2 * b + 1]["y_t"]
        out[b] = acc.T + bo_eff[None, :]
    return out


def kernel(**inputs):
    from concourse.bass_utils import run_bass_kernel_spmd

    inputs = {k: np.asarray(v) for k, v in inputs.items()}
    nc = get_program()
    in_maps, bo_eff = make_core_inputs(
        inputs["query"], inputs["key"], inputs["value"],
        inputs["Wq"], inputs["bq"], inputs["Wk"], inputs["bk"],
        inputs["Wv"], inputs["bv"], inputs["Wo"], inputs["bo"],
    )
    res = run_bass_kernel_spmd(nc, in_maps, list(range(8)))
    return combine_outputs(res.results, bo_eff)


# revision 24
# speedup vs baseline: 1.0327x; 1.0168x over previous
"""Multi-head attention (B=4, S=2048, D=1024, H=16, dk=64) on 8 TRN2 NeuronCores.

Sharding: core c = (batch b = c//2, head-group g = c%2 of 8 heads).
Each core computes its head-group's attention output and the partial output
projection (Wo rows for its heads); the host sums the two partials per batch
and adds the (folded) output bias.

Design (418us vs 474us baseline):
  - Everything bf16 on the PE (stationary + moving) -> FWL hides LDWEIGHTS,
    matmuls run at ~N*0.414ns at full p-state.
  - Scores psum is produced in the u' = 128*log2(e)*s domain (scale folded
    into Wq/bq on the host) so BOTH exp engines can consume it:
      * ScalarE activation Exp with scale=ln2/128 (free input affine)
      * a custom 8-stage DVE op EXP2_BITS_ANT writing uint16 bf16-BITS of
        2^(u'/128) directly: u2=u'-64; n1=(u2+M)-M (M=1.5*2^30: fp32 magic
        rounds to multiples of 128 -> floor); z=u2-n1 (=f1-64);
        out = u2 + (phi/128)*z^2 + (16320-32phi)
        (Schraudolph exp2 + parabolic mantissa correction; ~0.25% rms,
        ~0.1% after softmax normalization).
    Splitting exp 10/6 Scalar/DVE per head-pair removes the 273.7us
    single-engine exp wall that paced the baseline's attention phase; the
    attention loop then runs at the PE sequencer rate (~1.01us per skv
    tile = 4 matmuls + 4 ldweights).
  - AV weights padded to M=128 (vaug [skv,128]: V cols 0-63, ones col 64
    for denominators, junk cols 65-127 never read) keeping FWL on; the
    [128,512] f32 acc is still 1 PSUM bank.
  - y staged through ScalarE copies (Scalar has slack after the split).
  - Next-block Q-projection and prev-block out-projection are interleaved
    into the attention loop as fill chunks to keep the PE queue dense.
"""

import math

import numpy as np

B, S, D = 4, 2048, 1024
H, DK = 16, 64
LH = 8                 # heads per core
HK = LH * DK           # 512 (local concat dim)
BLK = 512              # Sq block size
NB = S // BLK          # 4
ST = S // 128          # 16 Skv tiles
KT = D // 128          # 8 contraction tiles over D
MT = HK // 128         # 4 m-tiles over local heads

# exp domain: psum holds u' = 128*log2(e) * scores
EXP_PHI = 0.34
EXP_C2 = 16320.0 - 32.0 * EXP_PHI
EXP_M = 1.5 * 2 ** 30
SCALAR_EXP_SCALE = math.log(2.0) / 128.0
U_SCALE = 128.0 / math.log(2.0)   # folded into Wq/bq (on top of 1/8)

# which i-tiles (per head-pair) run exp on the DVE instead of ScalarE.
# Spread so Scalar never runs more than 2 in a row: the two engines' exps
# must overlap or the exp chain paces the whole attention loop. Head-pairs
# 0-1 of each block also carry the out-projection psy->ysb copies on
# Scalar, so they shift more exp tiles to the DVE.
DVE_I_HEAVY = frozenset((1, 3, 5, 7, 9, 11, 13, 15))
DVE_I_LIGHT = frozenset((1, 4, 6, 9, 11, 14))

_CACHE = {}


def _register_dve_exp():
    """Register the EXP2_BITS_ANT custom DVE op (documented extension point:
    dve_ops.OPS + sub-opcode row; rows 17..31 are free in the 5-bit field)."""
    import concourse.dve_ops as DOPS
    from concourse.dve_spec import C0, C1, C2, C3, Spec, Src0, lower, sq
    from concourse.dve_spec import _spill_c3_to_src1
    from concourse.dve_uop import DveOpSpec

    name = "EXP2_BITS_ANT"
    for op in DOPS.OPS:
        if op.name == name:
            return op

    u2 = Src0 - C0            # u' - 64
    h = u2 + C1               # + M  (fp32 rounds to a multiple of 128)
    n1 = h - C1               # 128*floor(u'/128)
    z = u2 - n1               # f1 - 64
    t = sq(z) * C3            # (phi/128) * z^2      (C3 via in1 spill)
    body = (u2 + t) + C2      # u' - 64 + t + (16320-32phi)

    def _ref(in0, in1, c0, c1, c2):
        f = np.float32
        u = in0.astype(f)
        u2 = (u - f(c0)).astype(f)
        h = (u2 + f(c1)).astype(f)
        n1 = (h - f(c1)).astype(f)
        z = (u2 - n1).astype(f)
        t = (z * z * np.asarray(in1, f).reshape(-1, 1)).astype(f)
        return ((u2 + t) + f(c2)).astype(f)

    spec = Spec(body=_spill_c3_to_src1(body), reference=_ref)
    row = max(DOPS._SUB_OPCODE_FOR_NAME.values()) + 1
    assert row < 0x20
    DOPS._SUB_OPCODE_FOR_NAME[name] = row
    uops = lower(spec, ver="v3")
    sha = DveOpSpec(name=name, opcode=row, uops=uops, rd1_en=True).sha("v3")
    op = DOPS.DveOp(name, spec, subdim=False, uops_sha={"v3": sha})
    DOPS.OPS.append(op)
    DOPS.CUSTOM_DVE_SPECS[name] = spec
    return op


def _build_program():
    from contextlib import ExitStack
    import concourse.tile as tile
    from concourse import bacc, mybir

    exp_op = _register_dve_exp()

    f32 = mybir.dt.float32
    bf16 = mybir.dt.bfloat16
    u16 = mybir.dt.uint16
    Exp = mybir.ActivationFunctionType.Exp

    nc = bacc.Bacc("TRN2", target_bir_lowering=False, debug=False, num_devices=8)

    xq_d = nc.dram_tensor("xq_t", [D, S], bf16, kind="ExternalInput")
    xk_d = nc.dram_tensor("xk_t", [D, S], bf16, kind="ExternalInput")
    xv_d = nc.dram_tensor("xv_t", [D, S], bf16, kind="ExternalInput")
    wq_d = nc.dram_tensor("wq", [D, HK], bf16, kind="ExternalInput")
    wk_d = nc.dram_tensor("wk", [D, HK], bf16, kind="ExternalInput")
    wv_d = nc.dram_tensor("wv", [D, HK], bf16, kind="ExternalInput")
    wo_d = nc.dram_tensor("wo", [HK, D], bf16, kind="ExternalInput")
    bq_d = nc.dram_tensor("bq2", [128, MT], f32, kind="ExternalInput")
    bk_d = nc.dram_tensor("bk2", [128, MT], f32, kind="ExternalInput")
    y_d = nc.dram_tensor("y_t", [D, S], f32, kind="ExternalOutput")

    with tile.TileContext(nc) as tc, ExitStack() as ctx:
        big = ctx.enter_context(tc.tile_pool(name="big", bufs=1))
        xs = ctx.enter_context(tc.tile_pool(name="xs", bufs=3))
        es_pool = ctx.enter_context(tc.tile_pool(name="es", bufs=12))
        ot_pool = ctx.enter_context(tc.tile_pool(name="ot", bufs=2))
        rpool = ctx.enter_context(tc.tile_pool(name="r", bufs=3))
        upool = ctx.enter_context(tc.tile_pool(name="u", bufs=3))
        ypool = ctx.enter_context(tc.tile_pool(name="y", bufs=3))
        # PSUM: psS 2x[128,1024] (4 banks) + psW 4x[128,512] (4 banks) = 8
        psS = ctx.enter_context(tc.tile_pool(name="psS", bufs=2, space="PSUM"))
        psW = ctx.enter_context(tc.tile_pool(name="psW", bufs=4, space="PSUM"))

        bq_sb = big.tile([128, MT], f32)
        bk_sb = big.tile([128, MT], f32)
        nc.sync.dma_start(bq_sb[:], bq_d[:])
        nc.sync.dma_start(bk_sb[:], bk_d[:])
        phi_sb = big.tile([128, 1], f32)
        nc.vector.memset(phi_sb[:], EXP_PHI / 128.0)

        qt = big.tile([128, MT, S], bf16)
        kt_ = big.tile([128, MT, S], bf16)
        # vaug[skv, st, h, 0:64] = V, col 64 = ones (denominator row), cols
        # 65:128 = junk-ones (never read; the pad keeps NumWeights==128 so
        # FWL stays enabled for the AV matmuls)
        vaug = big.tile([128, ST, LH, 128], bf16)
        nc.vector.memset(vaug[:, :, :, :].bitcast(u16), 0x3F80)

        # weight layouts keep each [128,128] matmul slice contiguous in the
        # innermost dim so Fast Weight Load stays enabled
        wq_sb = big.tile([128, MT, KT, 128], bf16, name="wq_sb")
        wk_sb = big.tile([128, MT, KT, 128], bf16, name="wk_sb")
        wv_sb = big.tile([128, KT, HK], bf16, name="wv_sb")
        wo_sb = big.tile([128, KT, MT, 128], bf16, name="wo_sb")

        W2 = 2 * BLK   # 1024-wide projection chains (2 Sq blocks per psum)

        def dma_x2(x_dram, half, tag, split=False):  # noqa: doc
            # one [128, KT, 1024] bf16 tile = half the sequence
            xt = xs.tile([128, KT, W2], bf16, tag="xs", name=f"xt_{tag}{half}")
            if split:  # stripe the cold-start tile across DMA queues
                for kt in range(KT):
                    nc.sync.dma_start(
                        xt[:, kt, :],
                        x_dram.ap()[kt * 128 : (kt + 1) * 128,
                                    half * W2 : (half + 1) * W2],
                    )
            else:
                nc.sync.dma_start(
                    xt[:],
                    x_dram.ap()[:, half * W2 : (half + 1) * W2]
                    .rearrange("(kt p) s -> p kt s", p=128),
                )
            return xt

        def proj_half(xt, w_sb, bias_sb, dst, half, mt, tag):
            # two N=512 chains sharing one [128,1024] psum tile (matmul
            # output must stay within one PSUM bank) + a single wide drain
            pp = psS.tile([128, W2], f32, tag="psS", name=f"pp_{tag}{half}_{mt}")
            for kt in range(KT):
                for h2 in range(2):
                    nc.tensor.matmul(
                        pp[:, h2 * BLK : (h2 + 1) * BLK],
                        w_sb[:, mt, kt, :],
                        xt[:, kt, h2 * BLK : (h2 + 1) * BLK],
                        start=(kt == 0),
                        stop=(kt == KT - 1),
                        skip_group_check=True,
                    )
            nc.vector.tensor_scalar_add(
                dst[:, mt, half * W2 : (half + 1) * W2], pp[:],
                bias_sb[:, mt : mt + 1],
            )

        # ---- K projection, full Q projection, V projection (N=1024) ----
        xk0 = dma_x2(xk_d, 0, "k", split=True)
        wk_re = wk_d.ap().rearrange("(kt p) (mt m) -> p mt kt m", p=128, mt=MT, m=128)
        for mt in range(MT):
            for kh in range(2):
                nc.sync.dma_start(wk_sb[:, mt, kh * 4 : (kh + 1) * 4],
                                  wk_re[:, mt, kh * 4 : (kh + 1) * 4])
        xk1 = dma_x2(xk_d, 1, "k")
        nc.sync.dma_start(
            wq_sb[:],
            wq_d.ap().rearrange("(kt p) (mt m) -> p mt kt m", p=128, mt=MT, m=128),
        )
        for mt in range(MT):
            proj_half(xk0, wk_sb, bk_sb, kt_, 0, mt, "k")
        xq0 = dma_x2(xq_d, 0, "q")
        for mt in range(MT):
            proj_half(xk1, wk_sb, bk_sb, kt_, 1, mt, "k")
        xq1 = dma_x2(xq_d, 1, "q")
        nc.sync.dma_start(wv_sb[:], wv_d.ap().rearrange("(kt p) m -> p kt m", p=128))
        for mt in range(MT):
            proj_half(xq0, wq_sb, bq_sb, qt, 0, mt, "q")
        xv0 = dma_x2(xv_d, 0, "v")
        for mt in range(MT):
            proj_half(xq1, wq_sb, bq_sb, qt, 1, mt, "q")
        nc.sync.dma_start(
            wo_sb[:],
            wo_d.ap().rearrange("(kt p) (mo m) -> p mo kt m", p=128, kt=MT, m=128),
        )
        xv1 = dma_x2(xv_d, 1, "v")
        for half, xtv in ((0, xv0), (1, xv1)):
            for qp in range(4):
                pp = [psW.tile([128, BLK], f32, tag="psW",
                               name=f"pp_v{half}_{qp}_{t}") for t in range(2)]
                for kt in range(KT):
                    for t in range(2):
                        q = 2 * qp + t
                        nc.tensor.matmul(
                            pp[t][:],
                            xtv[:, kt, q * 128 : (q + 1) * 128],
                            wv_sb[:, kt, :],
                            start=(kt == 0),
                            stop=(kt == KT - 1),
                            skip_group_check=True,
                        )
                for t in range(2):
                    st = half * 8 + 2 * qp + t
                    nc.vector.tensor_copy(
                        vaug[:, st, :, 0:DK],
                        pp[t][:].rearrange("p (h k) -> p h k", h=LH),
                    )

        # ---- fill queue: PE chunks interleaved into the attention loop ----
        fills = []

        def emit_outproj_chunks(jprev, ot_prev):
            for mo in range(KT):
                def op_(mo=mo, jprev=jprev, ot_prev=ot_prev):
                    psy = psW.tile([128, BLK], f32, tag="psW",
                                   name=f"psy{jprev}_{mo}")
                    for kt in range(MT):
                        nc.tensor.matmul(
                            psy[:],
                            wo_sb[:, mo, kt, :],
                            ot_prev[:, kt, :],
                            start=(kt == 0),
                            stop=(kt == MT - 1),
                            skip_group_check=True,
                        )
                    ysb = ypool.tile([128, BLK], f32, tag="y",
                                     name=f"ysb{jprev}_{mo}")
                    if mo % 2 == 0:
                        nc.scalar.copy(ysb[:], psy[:])
                    else:
                        nc.vector.tensor_copy(ysb[:], psy[:])
                    if jprev == NB - 1:
                        for hy in range(2):
                            nc.sync.dma_start(
                                y_d[mo * 128 : (mo + 1) * 128,
                                    jprev * BLK + hy * 256
                                    : jprev * BLK + (hy + 1) * 256],
                                ysb[:, hy * 256 : (hy + 1) * 256],
                            )
                    else:
                        nc.sync.dma_start(
                            y_d[mo * 128 : (mo + 1) * 128,
                                jprev * BLK : (jprev + 1) * BLK], ysb[:]
                        )
                fills.append(op_)

        def fill(n):
            for _ in range(n):
                if fills:
                    fills.pop(0)()

        # ---- attention: per block, per head-pair, i-major; AV one-late ----
        ot_prev = None
        for j in range(NB):
            otj = ot_pool.tile([128, MT, BLK], bf16)
            if j >= 1:
                emit_outproj_chunks(j - 1, ot_prev)
            for hp in range(LH // 2):
                mt = hp
                acc = []

                def emit_av(iu, es_t, acc=acc, hp=hp, j=j):
                    if not acc:
                        acc.extend(psW.tile([128, BLK], f32, tag="psW",
                                            name=f"acc{j}_{hp}_{p2}")
                                   for p2 in range(2))
                    for pi in range(2):
                        h = 2 * hp + pi
                        nc.tensor.matmul(
                            acc[pi][:],
                            vaug[:, iu, h, :],
                            es_t[:, pi * BLK : (pi + 1) * BLK],
                            start=(iu == 0),
                            stop=(iu == ST - 1),
                            skip_group_check=True,
                        )

                # i-tiles processed in PAIRS: both scores pairs (64x128
                # row-tiled mode) back-to-back, then a batch of AV matmuls
                # (128x128 mode) — halves the PE tiling-mode switch drains
                es_q = []
                for i in range(ST):
                    ps2 = psS.tile([128, 2 * BLK], f32, tag="psS",
                                   name=f"ps_s{j}_{hp}_{i}")
                    for pi in range(2):
                        bp = pi * 64
                        nc.tensor.matmul(
                            ps2[:, pi * BLK : (pi + 1) * BLK],
                            kt_[bp : bp + 64, mt, i * 128 : (i + 1) * 128],
                            qt[bp : bp + 64, mt, j * BLK : (j + 1) * BLK],
                            start=True,
                            stop=True,
                            skip_group_check=True,
                        )
                    es = es_pool.tile([128, 2 * BLK], bf16, tag="es")
                    if i in DVE_I_LIGHT:
                        nc.vector._custom_dve(
                            exp_op, out=es[:].bitcast(u16), in0=ps2[:],
                            in1=phi_sb[:, 0:1], s0=64.0, s1=EXP_M, imm2=EXP_C2,
                        )
                    else:
                        nc.scalar.activation(es[:], ps2[:], Exp,
                                             scale=SCALAR_EXP_SCALE)
                    es_q.append((i, es))
                    if i % 2 == 1:
                        while len(es_q) > 2:
                            iu, es_t = es_q.pop(0)
                            emit_av(iu, es_t)
                        if i in (5, 9):
                            fill(1)
                for iu, es_t in es_q:
                    emit_av(iu, es_t)
                def emit_norm(acc=acc, mt=mt, hp=hp, j=j, otj=otj):
                    rrow, rf, rbc = [], [], []
                    for pi in range(2):
                        rrow.append(rpool.tile([1, BLK], f32, tag="r",
                                               name=f"rr{j}_{hp}_{pi}"))
                        nc.vector.tensor_copy(rrow[pi][:],
                                              acc[pi][DK : DK + 1, :])
                    for pi in range(2):
                        rf.append(rpool.tile([1, BLK], f32, tag="rf",
                                             name=f"rf{j}_{hp}_{pi}"))
                        nc.vector.reciprocal_approx_fast(rf[pi][:], rrow[pi][:])
                    for pi in range(2):
                        rbc.append(upool.tile([DK, BLK], f32, tag="rb",
                                              name=f"rb{j}_{hp}_{pi}"))
                        nc.gpsimd.partition_broadcast(rbc[pi][:], rf[pi][:])
                    for pi in range(2):
                        nc.vector.tensor_mul(otj[pi * 64 : pi * 64 + 64, mt, :],
                                             acc[pi][0:DK, :], rbc[pi][:])
                emit_norm()
            fill(len(fills))
            ot_prev = otj
        emit_outproj_chunks(NB - 1, ot_prev)
        fill(len(fills))

    nc.compile()
    return nc


def get_program():
    if "nc" not in _CACHE:
        _CACHE["nc"] = _build_program()
    return _CACHE["nc"]


def make_core_inputs(query, key, value, Wq, bq, Wk, bk, Wv, bv, Wo, bo):
    """Build the 8 per-core input dicts (and the folded output bias)."""
    import ml_dtypes
    f = np.float32
    bf = ml_dtypes.bfloat16
    # scores scale 1/8 and the exp2 domain 128*log2(e) folded into Wq/bq
    qs = U_SCALE / 8.0
    in_maps = []
    for c in range(8):
        b, g = c // 2, c % 2
        hs = slice(g * LH, (g + 1) * LH)
        m = {
            "xq_t": np.ascontiguousarray(query[b].T).astype(bf),
            "xk_t": np.ascontiguousarray(key[b].T).astype(bf),
            "xv_t": np.ascontiguousarray(value[b].T).astype(bf),
            "wq": np.ascontiguousarray(
                Wq[hs].transpose(1, 0, 2).reshape(D, HK) * qs
            ).astype(bf),
            "wk": np.ascontiguousarray(
                Wk[hs].transpose(1, 0, 2).reshape(D, HK)
            ).astype(bf),
            "wv": np.ascontiguousarray(
                Wv[hs].transpose(1, 0, 2).reshape(D, HK)
            ).astype(bf),
            "wo": np.ascontiguousarray(Wo[g * HK : (g + 1) * HK, :]).astype(bf),
            "bq2": np.ascontiguousarray(
                (bq[hs].reshape(HK) * qs).reshape(MT, 128).T, dtype=f
            ),
            "bk2": np.ascontiguousarray(
                bk[hs].reshape(HK).reshape(MT, 128).T, dtype=f
            ),
        }
        in_maps.append(m)
    bo_eff = (bv.reshape(H * DK).astype(np.float64) @ Wo.astype(np.float64)
              + bo.astype(np.float64)).astype(f)
    return in_maps, bo_eff


def combine_outputs(results, bo_eff):
    """results: list of 8 dicts with 'y_t' [D, S]. Returns [B, S, D] f32."""
    out = np.empty((B, S, D), dtype=np.float32)
    for b in range(B):
        acc = results[2 * b]["y_t"] + results[2 * b + 1]["y_t"]
        out[b] = acc.T + bo_eff[None, :]
    return out


def kernel(**inputs):
    from concourse.bass_utils import run_bass_kernel_spmd

    inputs = {k: np.asarray(v) for k, v in inputs.items()}
    nc = get_program()
    in_maps, bo_eff = make_core_inputs(
        inputs["query"], inputs["key"], inputs["value"],
        inputs["Wq"], inputs["bq"], inputs["Wk"], inputs["bk"],
        inputs["Wv"], inputs["bv"], inputs["Wo"], inputs["bo"],
    )
    res = run_bass_kernel_spmd(nc, in_maps, list(range(8)))
    return combine_outputs(res.results, bo_eff)
